# revision 1
# baseline (speedup 1.0000x reference)
"""Trainium2 Bass kernel for nn_Evaluate_66735201845638.

Stereo-matching style op: bilinear-sample right_features at K=10 per-pixel
(offset_x, offset_y) candidates, L1-compare against left_features over C=32
channels, sharp softmax (T=10000) over K, output expectation of the offsets.

Strategy (8 cores, rows sharded, 32 rows each):
  - Host: slices per-core inputs, reorders right_features into even/odd
    pixel-PAIR buffers [pairs, 64f32] (256B elements) over a 63-row halo
    window, computes the int16 gather indices (pure addressing) in the
    HW-wrapped layout, and folds the coordinate grids into the offsets
    (offxf = x - offset_x, offyf = (y - win_start) - offset_y).
  - Device: computes fractional weights, dma_gather's the two
    y-corner-row pair elements per (k,w) location (each 256B = 2 pixels x
    32 channels), lerps in y then in x, subtracts left, |.|-reduces over
    channels, softmax over K, and the weighted offset sums.
  - Host: ox = x - dev_x, oy = y - dev_y, stitch cores.

Self-contained: hardcodes B=1, C=32, H=256, W=512, K=10, 8 cores.
"""

import numpy as np

B, C, H, W, K = 1, 32, 256, 512, 10
NCORES = 8
HLOC = H // NCORES            # 32 output rows per core
MARGIN = 15                   # halo rows above/below (|offset_y| <= 14.5 safe)
WIN = HLOC + 2 * MARGIN + 1   # 63-row gather window
NPAIR = W // 2                # 256 pairs per row per parity
PROWS = WIN                   # 63 patch rows (r = y0_loc + 1 in [0, 62])
NELEM = 2 * PROWS * NPAIR     # even-parity patches + odd-parity patches
WC = W // 128                 # 4 column chunks of 128
NI = K * W                    # 5120 gather indices per (row, corner)
TEMP_SCALE = -10000.0 / C     # strength = -T/C * sum_c|diff|

_cache = {}


def _build_bass():
    import concourse.bass as bass
    import concourse.bacc as bacc
    import concourse.tile as tile
    import concourse.mybir as mybir
    from concourse.mybir import AluOpType as alu

    dt = mybir.dt
    nc = bacc.Bacc("TRN2", target_bir_lowering=False, num_devices=NCORES)

    rightw = nc.dram_tensor("rightw", [NELEM, 128], dt.float32, kind="ExternalInput")
    leftt = nc.dram_tensor("leftt", [128, HLOC * WC * C], dt.float32, kind="ExternalInput")
    offxf = nc.dram_tensor("offxf", [128, HLOC * K * WC], dt.float32, kind="ExternalInput")
    offyf = nc.dram_tensor("offyf", [128, HLOC * K * WC], dt.float32, kind="ExternalInput")
    fxw = nc.dram_tensor("fxw", [128, HLOC * K * WC], dt.float32, kind="ExternalInput")
    fyw = nc.dram_tensor("fyw", [128, HLOC * K * WC], dt.float32, kind="ExternalInput")
    gidx = nc.dram_tensor("gidx", [128, HLOC * (NI // 16)], dt.int16, kind="ExternalInput")
    outx = nc.dram_tensor("outx", [128, HLOC * WC], dt.float32, kind="ExternalOutput")
    outy = nc.dram_tensor("outy", [128, HLOC * WC], dt.float32, kind="ExternalOutput")

    F = HLOC * K * WC  # 1280

    def vw(sl, dims):
        """AP view: keep slice's partition dim + offset, replace free dims."""
        return bass.AP(tensor=sl.tensor, offset=sl.offset,
                       ap=[list(sl.ap[0])] + [list(d) for d in dims])

    with tile.TileContext(nc) as tc:
        with (
            tc.tile_pool(name="persist", bufs=1) as persist,
            tc.tile_pool(name="stream", bufs=2) as stream,
        ):
            offx_sb = persist.tile([128, F], dt.float32)
            offy_sb = persist.tile([128, F], dt.float32)
            left_sb = persist.tile([128, HLOC * WC * C], dt.float32)
            fx = persist.tile([128, F], dt.float32)
            fy = persist.tile([128, F], dt.float32)
            nc.sync.dma_start(out=offx_sb, in_=offxf.ap())
            nc.sync.dma_start(out=offy_sb, in_=offyf.ap())
            nc.sync.dma_start(out=left_sb, in_=leftt.ap())
            nc.sync.dma_start(out=fx, in_=fxw.ap())
            nc.sync.dma_start(out=fy, in_=fyw.ap())

            dist = persist.tile([128, F], dt.float32)  # layout h*40 + wc*10 + k

            rightw_ap = rightw.ap()

            for h in range(HLOC):
                gidx_sb = stream.tile([128, NI // 16], dt.int16, tag="gidx")
                nc.sync.dma_start(
                    out=gidx_sb,
                    in_=gidx.ap()[:, h * (NI // 16):(h + 1) * (NI // 16)],
                )
                # each 512B element is a 2x2 pixel patch x 32 channels:
                # [y0(x0,x0+1) | y1(x0,x0+1)], so one gather covers both rows
                G = stream.tile([128, K * WC, 128], dt.float32, tag="G")
                NIC = 1024  # dma_gather num_idxs hard limit (Q7 scratch)
                for c in range(NI // NIC):
                    nc.gpsimd.dma_gather(
                        out_ap=G[:, c * (NIC // 128):(c + 1) * (NIC // 128), :],
                        in_ap=rightw_ap,
                        idxs_ap=gidx_sb[:, c * (NIC // 16):(c + 1) * (NIC // 16)],
                        num_idxs=NIC,
                        num_idxs_reg=NIC,
                        elem_size=128,
                    )
                G0 = G[:, :, 0:64]
                G1 = G[:, :, 64:128]
                # y-lerp: yb = G0 + fy*(G1 - G0)   (zero-element makes masks exact)
                wyr = stream.tile([128, K * WC, 64], dt.float32, tag="wyr")
                nc.scalar.activation(
                    out=wyr[:, :, :],
                    in_=vw(fy[:, h * K * WC:(h + 1) * K * WC], [[1, K * WC], [0, 64]]),
                    func=mybir.ActivationFunctionType.Copy,
                )
                d = stream.tile([128, K * WC, 64], dt.float32, tag="d")
                nc.gpsimd.tensor_sub(d, G1, G0)
                nc.vector.tensor_mul(d, d, wyr)
                nc.vector.tensor_add(d, d, G0)
                # x-lerp on the two 32ch halves: sx = yb0 + fx*(yb1 - yb0)
                wxr = stream.tile([128, K * WC, C], dt.float32, tag="wxr")
                nc.scalar.activation(
                    out=wxr[:, :, :],
                    in_=vw(fx[:, h * K * WC:(h + 1) * K * WC], [[1, K * WC], [0, C]]),
                    func=mybir.ActivationFunctionType.Copy,
                )
                sx = stream.tile([128, K * WC, C], dt.float32, tag="sx")
                nc.vector.tensor_sub(sx, d[:, :, C:2 * C], d[:, :, 0:C])
                nc.vector.tensor_mul(sx, sx, wxr)
                nc.vector.tensor_add(sx, sx, d[:, :, 0:C])
                # e = sx - left   (left bcast over k)
                nc.vector.tensor_sub(
                    sx, sx,
                    vw(left_sb[:, h * WC * C:(h + 1) * WC * C], [[0, K], [C, WC], [1, C]]),
                )
                # dist[h*40 + wc*10 + k] = sum_c |e|
                nc.vector.tensor_reduce(
                    out=vw(dist[:, h * K * WC:(h + 1) * K * WC], [[1, K], [K, WC]]),
                    in_=sx[:, :, :],
                    axis=mybir.AxisListType.X,
                    op=alu.add,
                    apply_absolute_value=True,
                )

            # ---- softmax over K + weighted sums ----
            HW4 = HLOC * WC  # 128
            dist3 = vw(dist[:, :], [[K, HW4], [1, K]])
            m = persist.tile([128, HW4], dt.float32, tag="m")
            nc.vector.tensor_reduce(out=m[:, :], in_=dist3, axis=mybir.AxisListType.X, op=alu.min)
            p = persist.tile([128, F], dt.float32)
            nc.vector.tensor_sub(
                vw(p[:, :], [[K, HW4], [1, K]]), dist3,
                vw(m[:, :], [[1, HW4], [0, K]]),
            )
            # p = exp(TEMP_SCALE * (dist - m))
            nc.scalar.activation(out=p, in_=p, func=mybir.ActivationFunctionType.Exp, scale=TEMP_SCALE)
            s = persist.tile([128, HW4], dt.float32, tag="s")
            nc.vector.tensor_reduce(out=s[:, :], in_=vw(p[:, :], [[K, HW4], [1, K]]), axis=mybir.AxisListType.X, op=alu.add)
            r = persist.tile([128, HW4], dt.float32, tag="r")
            nc.vector.reciprocal(r[:, :], s[:, :])

            # offx_sb layout h*40+k*4+wc viewed as (h, wc, k) to match p
            offx_v = vw(offx_sb[:, :], [[K * WC, HLOC], [1, WC], [WC, K]])
            offy_v = vw(offy_sb[:, :], [[K * WC, HLOC], [1, WC], [WC, K]])
            p3 = vw(p[:, :], [[K * WC, HLOC], [K, WC], [1, K]])
            for off_v, nm in ((offx_v, "nx"), (offy_v, "ny")):
                n_t = persist.tile([128, F], dt.float32, tag="n_t")
                nc.vector.tensor_tensor(vw(n_t[:, :], [[K * WC, HLOC], [K, WC], [1, K]]), off_v, p3, op=alu.mult)
                acc = persist.tile([128, HW4], dt.float32, tag="acc")
                nc.vector.tensor_reduce(out=acc[:, :], in_=vw(n_t[:, :], [[K, HW4], [1, K]]), axis=mybir.AxisListType.X, op=alu.add)
                nc.vector.tensor_mul(acc, acc, r)
                nc.sync.dma_start(out=(outx if nm == "nx" else outy).ap(), in_=acc[:, :])

    nc.compile()
    return nc


def _host_prep(left_features, right_features, offset_x, offset_y):
    """Per-core input dicts. All layout/addressing on host; arithmetic on device."""
    lf = np.asarray(left_features, np.float32)
    rf = np.asarray(right_features, np.float32)
    ox = np.asarray(offset_x, np.float32)
    oy = np.asarray(offset_y, np.float32)
    r_hwc = np.ascontiguousarray(rf[0].transpose(1, 2, 0))  # [H, W, C]
    l_hwc = lf[0].transpose(1, 2, 0)                        # [H, W, C]
    xs = np.arange(W, dtype=np.float32)

    in_maps = []
    metas = []
    for ci in range(NCORES):
        h0 = ci * HLOC
        ws = min(max(h0 - MARGIN, 0), H - WIN)
        rows = slice(h0, h0 + HLOC)

        # 64 window rows [ws-1, ws+63); row ws-1 is zeros at the global top
        win64 = np.zeros((WIN + 1, W, C), np.float32)
        lo = max(ws - 1, 0)
        win64[lo - (ws - 1):] = r_hwc[lo:ws + WIN]
        PA = win64                                            # even-parity pixels
        PB = np.concatenate([np.zeros((WIN + 1, 1, C), np.float32), win64[:, :W - 1]], axis=1)
        rightw = np.empty((NELEM, 128), np.float32)
        for pi, P in ((0, PA), (1, PB)):
            P2 = P.reshape(WIN + 1, NPAIR, 64)
            patch = np.concatenate([P2[:-1], P2[1:]], axis=-1)  # [63, 256, 128]
            rightw[pi * PROWS * NPAIR:(pi + 1) * PROWS * NPAIR] = patch.reshape(-1, 128)

        # leftt [128, h*4wc*32c]
        leftt = np.ascontiguousarray(
            l_hwc[rows].reshape(HLOC, WC, 128, C).transpose(2, 0, 1, 3)
        ).reshape(128, -1)

        # folded offsets, [128, h*40 + k*4 + wc]
        oxf = xs[None, None, :] - ox[0, :, rows, :]                      # [K, 32, 512]
        hg = np.arange(h0, h0 + HLOC, dtype=np.float32)
        oyf = (hg[None, :, None] - ws) - oy[0, :, rows, :]
        def fold(a):
            return np.ascontiguousarray(
                a.reshape(K, HLOC, WC, 128).transpose(3, 1, 0, 2)
            ).reshape(128, -1)
        offxf_h = fold(oxf)
        offyf_h = fold(oyf)

        # gather indices (f32 math identical to device)
        rx = np.clip(oxf, 0.0, np.float32(W - 1))
        ixf = rx - np.float32(0.5)
        x0 = np.floor(ixf).astype(np.int32)                              # [-1, 510]
        fxh = (ixf - np.floor(ixf)).astype(np.float32)
        ry_loc = np.clip(oyf, np.float32(-ws), np.float32(H - 1 - ws))
        iyf = ry_loc - np.float32(0.5)
        y0 = np.floor(iyf).astype(np.int32)                              # window-local
        fyh = (iyf - np.floor(iyf)).astype(np.float32)
        par = x0 & 1
        e = (x0 + par) >> 1
        r = np.clip(y0, -1, PROWS - 2) + 1                               # patch row in [0, 62]
        idx0 = (par * PROWS + r) * NPAIR + e
        idx0 = np.clip(idx0, 0, NELEM - 1).astype(np.int16)

        # wrapped layout [16, h*10k*4wc*8g] replicated to 128 partitions
        gi = idx0.reshape(K, HLOC, WC, 8, 16).transpose(4, 1, 0, 2, 3)   # [16, 32, 10, 4, 8]
        gi = np.ascontiguousarray(gi).reshape(16, -1)
        gidx_h = np.tile(gi, (8, 1))

        in_maps.append({
            "rightw": rightw, "leftt": leftt,
            "offxf": offxf_h, "offyf": offyf_h,
            "fxw": fold(fxh), "fyw": fold(fyh), "gidx": gidx_h,
        })
        metas.append((h0, ws))
    return in_maps, metas


def _host_post(results, metas):
    ox = np.empty((1, 1, H, W), np.float32)
    oy = np.empty((1, 1, H, W), np.float32)
    xs = np.arange(W, dtype=np.float32)
    for ci, (res, (h0, ws)) in enumerate(zip(results, metas)):
        dx = res["outx"].reshape(128, HLOC, WC).transpose(1, 2, 0).reshape(HLOC, W)
        dy = res["outy"].reshape(128, HLOC, WC).transpose(1, 2, 0).reshape(HLOC, W)
        hg = np.arange(h0, h0 + HLOC, dtype=np.float32)
        ox[0, 0, h0:h0 + HLOC] = xs[None, :] - dx
        oy[0, 0, h0:h0 + HLOC] = (hg[:, None] - ws) - dy
    return ox, oy


def kernel(left_features, right_features, offset_x, offset_y):
    from concourse.bass_utils import run_bass_kernel_spmd

    assert left_features.shape == (B, C, H, W)
    in_maps, metas = _host_prep(left_features, right_features, offset_x, offset_y)
    if "nc" not in _cache:
        _cache["nc"] = _build_bass()
    res = run_bass_kernel_spmd(_cache["nc"], in_maps, core_ids=list(range(NCORES)))
    return _host_post(res.results, metas)



# revision 9
# speedup vs baseline: 1.5956x; 1.5956x over previous
"""Trainium2 Bass kernel for nn_Evaluate_66735201845638.

Stereo-matching op: bilinear-sample right_features at K=10 per-pixel
(offset_x, offset_y) candidates, L1-compare against left_features over C=32
channels, sharp softmax (T=10000) over K, output expectation of the offsets.

Strategy (8 cores, rows sharded, 32 rows each), v2 (fp16 compute):
  - Host: packs right_features (fp16) into 4 x-parity patch buffers of 512B
    elements [2 rows x 4 px x 32 ch] over a 63-row halo window, computes
    int16 gather indices + fp16 bilinear corner weights wa..wd inputs
    (fx, fy), fp16 left features, f32 raw offsets.
  - Device per output row h: two 2560-idx dma_gathers fetch the per-sample
    corner patches (fp16, no cast needed); Act broadcasts the 4 per-sample
    weights to channel width; DVE does the flat 4-corner weighted sum in
    fp16 (2x mode); Pool computes e = s - left with f32 output; DVE
    abs-reduces over channels into dist. Every 8 rows a chunked softmax
    over K produces the weighted offset sums.
  - Host: stitches per-core [32, 512] outputs.

Self-contained: hardcodes B=1, C=32, H=256, W=512, K=10, 8 cores.
"""

import numpy as np

B, C, H, W, K = 1, 32, 256, 512, 10
NCORES = 8
HLOC = H // NCORES            # 32 output rows per core
MARGIN = 15                   # halo rows above/below (|offset_y| <= 14.5 safe)
WIN = HLOC + 2 * MARGIN + 1   # 63-row gather window
PROWS = WIN                   # 63 patch rows (r = y0_loc + 1 in [0, 62])
NE = 130                      # elements per (parity, patch row)
WC = W // 128                 # 4 column chunks of 128
NI = K * W                    # 5120 gather indices per row
NIC = 1024                    # indices per dma_gather call (Q7 scratch limit)
F = HLOC * K * WC             # 1280
HW4 = HLOC * WC               # 128
CH = 8                        # rows per softmax chunk
NCH = HLOC // CH              # 4 chunks
TEMP_SCALE = -10000.0 / C

_cache = {}


def _build_bass():
    import concourse.bass as bass
    import concourse.bacc as bacc
    import concourse.tile as tile
    import concourse.mybir as mybir
    from concourse.mybir import AluOpType as alu

    dt = mybir.dt
    nc = bacc.Bacc("TRN2", target_bir_lowering=False, num_devices=NCORES)

    rightw = nc.dram_tensor("rightw", [4 * PROWS * NE, 256], dt.float16,
                            kind="ExternalInput")
    leftt = nc.dram_tensor("leftt", [128, HLOC * WC * C], dt.float16,
                           kind="ExternalInput")
    offx = nc.dram_tensor("offx", [128, F], dt.float16, kind="ExternalInput")
    offy = nc.dram_tensor("offy", [128, F], dt.float16, kind="ExternalInput")
    fxw = nc.dram_tensor("fxw", [128, F], dt.float16, kind="ExternalInput")
    fyw = nc.dram_tensor("fyw", [128, F], dt.float16, kind="ExternalInput")
    gidx = nc.dram_tensor("gidx", [128, HLOC * (NI // 16)], dt.int16,
                          kind="ExternalInput")
    outx = nc.dram_tensor("outx", [128, HW4], dt.float32, kind="ExternalOutput")
    outy = nc.dram_tensor("outy", [128, HW4], dt.float32, kind="ExternalOutput")

    def vw(sl, dims):
        """AP view: keep slice's partition dim + offset, replace free dims."""
        return bass.AP(tensor=sl.tensor, offset=sl.offset,
                       ap=[list(sl.ap[0])] + [list(d) for d in dims])

    GH = NI // 16   # 320 gidx columns per row

    with tile.TileContext(nc) as tc:
        with (
            tc.tile_pool(name="persist", bufs=1) as persist,
            tc.tile_pool(name="stream", bufs=2) as stream,
        ):
            # ---- static loads (gidx chunked so gathers can start early) ----
            gidx_sb = persist.tile([128, HLOC * GH], dt.int16)
            for c in range(NCH):
                nc.sync.dma_start(
                    out=gidx_sb[:, c * CH * GH:(c + 1) * CH * GH],
                    in_=gidx.ap()[:, c * CH * GH:(c + 1) * CH * GH])
            fx = persist.tile([128, F], dt.float16)
            fy = persist.tile([128, F], dt.float16)
            left_sb = persist.tile([128, HLOC * WC * C], dt.float16)
            offx_sb = persist.tile([128, F], dt.float16)
            offy_sb = persist.tile([128, F], dt.float16)
            nc.sync.dma_start(out=fx, in_=fxw.ap())
            nc.sync.dma_start(out=fy, in_=fyw.ap())
            nc.sync.dma_start(out=left_sb, in_=leftt.ap())
            nc.sync.dma_start(out=offx_sb, in_=offx.ap())
            nc.sync.dma_start(out=offy_sb, in_=offy.ap())

            # ---- per-sample corner weights (fp16, once) ----
            wd = persist.tile([128, F], dt.float16)
            wb = persist.tile([128, F], dt.float16)
            wcw = persist.tile([128, F], dt.float16)
            wa = persist.tile([128, F], dt.float16)
            uu = persist.tile([128, F], dt.float16)
            nc.vector.tensor_mul(wd, fx, fy)
            nc.vector.tensor_sub(wb, fx, wd)
            nc.vector.tensor_sub(wcw, fy, wd)
            nc.vector.tensor_scalar(out=uu, in0=fx, scalar1=-1.0, scalar2=1.0,
                                    op0=alu.mult, op1=alu.add)
            nc.vector.tensor_sub(wa, uu, wcw)

            dist = persist.tile([128, F], dt.float32)   # layout h*40 + k*4 + wc
            outx_sb = persist.tile([128, HW4], dt.float32)
            outy_sb = persist.tile([128, HW4], dt.float32)

            rightw_ap = rightw.ap()
            wtiles = (wa, wb, wcw, wd)

            for h in range(HLOC):
                # gather: 5 calls x 1024 idxs -> G[p, j=(k*4+wc), 256]
                G = stream.tile([128, K * WC, 256], dt.float16, tag="G")
                for c in range(NI // NIC):
                    nc.gpsimd.dma_gather(
                        out_ap=G[:, c * (NIC // 128):(c + 1) * (NIC // 128), :],
                        in_ap=rightw_ap,
                        idxs_ap=gidx_sb[:, h * GH + c * (NIC // 16):
                                        h * GH + (c + 1) * (NIC // 16)],
                        num_idxs=NIC,
                        num_idxs_reg=NIC,
                        elem_size=256,
                    )
                # Act: broadcast the 4 weights to channel width (fp16)
                wT = []
                for wi, wt in enumerate(wtiles):
                    wTt = stream.tile([128, K * WC, C], dt.float16, tag=f"wT{wi}")
                    nc.scalar.activation(
                        out=wTt,
                        in_=vw(wt[:, h * K * WC:(h + 1) * K * WC],
                               [[1, K * WC], [0, C]]),
                        func=mybir.ActivationFunctionType.Copy)
                    wT.append(wTt)
                # DVE: flat 4-corner weighted sum (all fp16 packed, 2x mode)
                mA = stream.tile([128, K * WC, C], dt.float16, tag="mA")
                mB = stream.tile([128, K * WC, C], dt.float16, tag="mB")
                sU = stream.tile([128, K * WC, C], dt.float16, tag="sU")
                sV = stream.tile([128, K * WC, C], dt.float16, tag="sV")
                ss = stream.tile([128, K * WC, C], dt.float16, tag="ss")
                nc.vector.tensor_mul(mA, G[:, :, 0:32], wT[0])
                nc.vector.tensor_mul(mB, G[:, :, 32:64], wT[1])
                nc.vector.tensor_add(sU, mA, mB)
                nc.vector.tensor_mul(mA, G[:, :, 128:160], wT[2])
                nc.vector.tensor_mul(mB, G[:, :, 160:192], wT[3])
                nc.vector.tensor_add(sV, mA, mB)
                nc.vector.tensor_add(ss, sU, sV)
                # DVE: e = s - left (fp16; Pool is saturated by gather dispatch)
                ee = stream.tile([128, K * WC, C], dt.float16, tag="ee")
                nc.vector.tensor_sub(
                    ee, ss,
                    vw(left_sb[:, h * WC * C:(h + 1) * WC * C],
                       [[0, K], [C, WC], [1, C]]))
                # DVE: dist[h*40 + j] = sum_c |e|
                nc.vector.tensor_reduce(
                    out=vw(dist[:, h * K * WC:(h + 1) * K * WC], [[1, K * WC]]),
                    in_=ee, axis=mybir.AxisListType.X, op=alu.add,
                    apply_absolute_value=True)

                # ---- chunked softmax over K + weighted sums ----
                if h % CH == CH - 1:
                    c0 = (h // CH) * CH * K * WC        # dist col offset
                    o0 = (h // CH) * CH * WC            # out col offset
                    dv = vw(dist[:, c0:c0 + CH * K * WC],
                            [[K * WC, CH], [1, WC], [WC, K]])
                    mt = stream.tile([128, CH * WC], dt.float32, tag="mt")
                    nc.vector.tensor_reduce(
                        out=vw(mt[:, :], [[WC, CH], [1, WC]]), in_=dv,
                        axis=mybir.AxisListType.X, op=alu.min)
                    q = stream.tile([128, CH * WC * K], dt.float32, tag="q")
                    qv = vw(q[:, :], [[WC * K, CH], [K, WC], [1, K]])
                    nc.vector.tensor_tensor(
                        qv, dv, vw(mt[:, :], [[WC, CH], [1, WC], [0, K]]),
                        op=alu.subtract)
                    pt = stream.tile([128, CH * WC * K], dt.float32, tag="pt")
                    nc.scalar.activation(out=pt, in_=q,
                                         func=mybir.ActivationFunctionType.Exp,
                                         scale=TEMP_SCALE)
                    ptv = vw(pt[:, :], [[WC * K, CH], [K, WC], [1, K]])
                    st = stream.tile([128, CH * WC], dt.float32, tag="st")
                    nc.vector.tensor_reduce(
                        out=vw(st[:, :], [[WC, CH], [1, WC]]), in_=ptv,
                        axis=mybir.AxisListType.X, op=alu.add)
                    rec = stream.tile([128, CH * WC], dt.float32, tag="rec")
                    nc.vector.reciprocal(rec, st)
                    for off_sb, osb, tg in ((offx_sb, outx_sb, "x"),
                                            (offy_sb, outy_sb, "y")):
                        ov = vw(off_sb[:, c0:c0 + CH * K * WC],
                                [[K * WC, CH], [1, WC], [WC, K]])
                        tx = stream.tile([128, CH * WC * K], dt.float32,
                                         tag=f"tx{tg}")
                        nc.vector.tensor_tensor(
                            vw(tx[:, :], [[WC * K, CH], [K, WC], [1, K]]),
                            ov, ptv, op=alu.mult)
                        nx = stream.tile([128, CH * WC], dt.float32,
                                         tag=f"nx{tg}")
                        nc.vector.tensor_reduce(
                            out=vw(nx[:, :], [[WC, CH], [1, WC]]),
                            in_=vw(tx[:, :], [[WC * K, CH], [K, WC], [1, K]]),
                            axis=mybir.AxisListType.X, op=alu.add)
                        nc.vector.tensor_mul(osb[:, o0:o0 + CH * WC], nx, rec)

            nc.sync.dma_start(out=outx.ap(), in_=outx_sb[:, :])
            nc.sync.dma_start(out=outy.ap(), in_=outy_sb[:, :])

    nc.compile()
    return nc


def _host_prep(left_features, right_features, offset_x, offset_y):
    """Per-core input dicts. All layout/addressing on host; lerp on device."""
    lf = np.asarray(left_features, np.float32)
    rf = np.asarray(right_features, np.float32)
    ox = np.asarray(offset_x, np.float32)
    oy = np.asarray(offset_y, np.float32)
    r_hwc = np.ascontiguousarray(rf[0].transpose(1, 2, 0))  # [H, W, C]
    l_hwc = lf[0].transpose(1, 2, 0)                        # [H, W, C]
    xs = np.arange(W, dtype=np.float32)

    in_maps = []
    metas = []
    for ci in range(NCORES):
        h0 = ci * HLOC
        ws = min(max(h0 - MARGIN, 0), H - WIN)
        rows = slice(h0, h0 + HLOC)

        # 64 window rows [ws-1, ws+63); row ws-1 is zeros at the global top
        win64 = np.zeros((WIN + 1, W, C), np.float32)
        lo = max(ws - 1, 0)
        win64[lo - (ws - 1):] = r_hwc[lo:ws + WIN]
        # fp16 padded image, cols -4..518; 4-parity patch buffers of
        # [2 rows x 4 px x 32 ch] elements, col_start = pi + 4e - 4
        pad = np.zeros((WIN + 1, 523, C), np.float16)
        pad[:, 4:4 + W] = win64.astype(np.float16)
        rightw = np.empty((4, PROWS, NE, 256), np.float16)
        for pi in range(4):
            Vp = pad[:, pi:pi + 4 * NE].reshape(WIN + 1, NE, 4, C)
            rightw[pi] = np.concatenate([Vp[:-1], Vp[1:]], axis=2).reshape(
                PROWS, NE, 256)
        rightw = rightw.reshape(-1, 256)

        # leftt [128, h*128 + wc*32 + c] fp16
        leftt = np.ascontiguousarray(
            l_hwc[rows].astype(np.float16).reshape(HLOC, WC, 128, C)
            .transpose(2, 0, 1, 3)).reshape(128, -1)

        # coords (f32 math identical to reference)
        oxs = ox[0, :, rows, :]
        oys = oy[0, :, rows, :]
        rx = np.clip(xs[None, None, :] - oxs, 0.0, np.float32(W - 1))
        hg = np.arange(h0, h0 + HLOC, dtype=np.float32)
        ry_loc = np.clip((hg[None, :, None] - ws) - oys,
                         np.float32(-ws), np.float32(H - 1 - ws))
        ixf = rx - np.float32(0.5)
        x0 = np.floor(ixf).astype(np.int32)                  # [-1, 510]
        fxh = (ixf - np.floor(ixf)).astype(np.float32)
        iyf = ry_loc - np.float32(0.5)
        y0 = np.floor(iyf).astype(np.int32)                  # window-local
        fyh = (iyf - np.floor(iyf)).astype(np.float32)
        r = np.clip(y0, -1, PROWS - 2) + 1                   # patch row [0, 62]
        pi = x0 & 3
        e = (x0 >> 2) + 1
        idx0 = ((pi * PROWS + r) * NE + e).astype(np.int16)  # [K, HLOC, W]

        def fold(a, dtp):
            return np.ascontiguousarray(
                a.reshape(K, HLOC, WC, 128).transpose(3, 1, 0, 2)
            ).reshape(128, -1).astype(dtp)

        # wrapped gidx layout [16, h, k, wc, g] replicated to 128 partitions
        gi = idx0.reshape(K, HLOC, WC, 8, 16).transpose(4, 1, 0, 2, 3)
        gi = np.ascontiguousarray(gi).reshape(16, -1)
        gidx_h = np.tile(gi, (8, 1))

        in_maps.append({
            "rightw": rightw, "leftt": leftt,
            "offx": fold(oxs, np.float16), "offy": fold(oys, np.float16),
            "fxw": fold(fxh, np.float16), "fyw": fold(fyh, np.float16),
            "gidx": gidx_h,
        })
        metas.append((h0, ws))
    return in_maps, metas


def _host_post(results, metas):
    ox = np.empty((1, 1, H, W), np.float32)
    oy = np.empty((1, 1, H, W), np.float32)
    for res, (h0, ws) in zip(results, metas):
        # outx free layout: chunk*32 + hh*4 + wc, partition = w % 128
        dx = res["outx"].reshape(128, NCH, CH, WC).transpose(1, 2, 3, 0)
        dy = res["outy"].reshape(128, NCH, CH, WC).transpose(1, 2, 3, 0)
        ox[0, 0, h0:h0 + HLOC] = dx.reshape(HLOC, W)
        oy[0, 0, h0:h0 + HLOC] = dy.reshape(HLOC, W)
    return ox, oy


def kernel(left_features, right_features, offset_x, offset_y):
    from concourse.bass_utils import run_bass_kernel_spmd

    assert left_features.shape == (B, C, H, W)
    in_maps, metas = _host_prep(left_features, right_features,
                                offset_x, offset_y)
    if "nc" not in _cache:
        _cache["nc"] = _build_bass()
    res = run_bass_kernel_spmd(_cache["nc"], in_maps, core_ids=list(range(NCORES)))
    return _host_post(res.results, metas)


# revision 11
# speedup vs baseline: 1.7216x; 1.0790x over previous
"""Trainium2 Bass kernel for nn_Evaluate_66735201845638.

Stereo-matching op: bilinear-sample right_features at K=10 per-pixel
(offset_x, offset_y) candidates, L1-compare against left_features over C=32
channels, sharp softmax (T=10000) over K, output expectation of the offsets.

Strategy (8 cores, rows sharded, 32 rows each), v3 (fp16 compute):
  - Host: packs right_features (fp16) into 4 x-parity patch buffers of 512B
    elements [2 rows x 4 px x 32 ch] over a 63-row halo window, computes
    int16 gather indices, fp16 lerp fractions, fp16 left features and
    offsets.
  - Device per output row h: five 1024-idx dma_gathers (Q7 scratch limit)
    fetch the per-sample corner patches (fp16, no cast needed); DVE builds
    interleaved corner weights once at startup; Act broadcasts them to
    channel width; DVE does the 4-corner weighted sum in fp16 2x mode with
    a single fused multiply over all corners, then abs-reduces over
    channels into dist. Every 8 rows a chunked softmax over K produces the
    weighted offset sums.
  - Host: stitches per-core [32, 512] outputs.

Self-contained: hardcodes B=1, C=32, H=256, W=512, K=10, 8 cores.
"""

import numpy as np

B, C, H, W, K = 1, 32, 256, 512, 10
NCORES = 8
HLOC = H // NCORES            # 32 output rows per core
MARGIN = 15                   # halo rows above/below (|offset_y| <= 14.5 safe)
WIN = HLOC + 2 * MARGIN + 1   # 63-row gather window
PROWS = WIN                   # 63 patch rows (r = y0_loc + 1 in [0, 62])
NE = 130                      # elements per (parity, patch row)
WC = W // 128                 # 4 column chunks of 128
NI = K * W                    # 5120 gather indices per row
NIC = 1024                    # indices per dma_gather call (Q7 scratch limit)
F = HLOC * K * WC             # 1280
J = K * WC                    # 40 sample groups per row
HW4 = HLOC * WC               # 128
CH = 8                        # rows per softmax chunk
NCH = HLOC // CH              # 4 chunks
TEMP_SCALE = -10000.0 / C

_cache = {}


def _build_bass():
    import concourse.bass as bass
    import concourse.bacc as bacc
    import concourse.tile as tile
    import concourse.mybir as mybir
    from concourse.mybir import AluOpType as alu

    dt = mybir.dt
    nc = bacc.Bacc("TRN2", target_bir_lowering=False, num_devices=NCORES)

    rightw = nc.dram_tensor("rightw", [4 * PROWS * NE, 256], dt.float16,
                            kind="ExternalInput")
    leftt = nc.dram_tensor("leftt", [128, HLOC * WC * C], dt.float16,
                           kind="ExternalInput")
    offx = nc.dram_tensor("offx", [128, F], dt.float16, kind="ExternalInput")
    offy = nc.dram_tensor("offy", [128, F], dt.float16, kind="ExternalInput")
    fxw = nc.dram_tensor("fxw", [128, F], dt.float16, kind="ExternalInput")
    fyw = nc.dram_tensor("fyw", [128, F], dt.float16, kind="ExternalInput")
    gidx = nc.dram_tensor("gidx", [128, HLOC * (NI // 16)], dt.int16,
                          kind="ExternalInput")
    outx = nc.dram_tensor("outx", [128, HW4], dt.float32, kind="ExternalOutput")
    outy = nc.dram_tensor("outy", [128, HW4], dt.float32, kind="ExternalOutput")

    def vw(sl, dims):
        """AP view: keep slice's partition dim + offset, replace free dims."""
        return bass.AP(tensor=sl.tensor, offset=sl.offset,
                       ap=[list(sl.ap[0])] + [list(d) for d in dims])

    GH = NI // 16   # 320 gidx columns per row

    with tile.TileContext(nc) as tc:
        with (
            tc.tile_pool(name="persist", bufs=1) as persist,
            tc.tile_pool(name="stream", bufs=2) as stream,
            tc.tile_pool(name="gstream", bufs=3) as gstream,
        ):
            fx = persist.tile([128, F], dt.float16)
            fy = persist.tile([128, F], dt.float16)
            left_sb = persist.tile([128, HLOC * WC * C], dt.float16)
            offx_sb = persist.tile([128, F], dt.float16)
            offy_sb = persist.tile([128, F], dt.float16)
            nc.sync.dma_start(out=fx, in_=fxw.ap())
            nc.sync.dma_start(out=fy, in_=fyw.ap())
            nc.sync.dma_start(out=left_sb, in_=leftt.ap())
            nc.sync.dma_start(out=offx_sb, in_=offx.ap())
            nc.sync.dma_start(out=offy_sb, in_=offy.ap())

            # ---- interleaved corner weights wquad[j*4 + pair*2 + half] ----
            # order per sample j: [wa, wb, wc, wd] (y0x0, y0x1, y1x0, y1x1)
            wquad = persist.tile([128, 4 * F], dt.float16)
            uu = persist.tile([128, F], dt.float16)

            def wq(pos):
                return vw(wquad[:, pos:pos + 4 * F - 3], [[4, F]])

            nc.vector.tensor_tensor(wq(3), fx, fy, op=alu.mult)     # wd = fx*fy
            nc.vector.tensor_tensor(wq(1), fx, wq(3), op=alu.subtract)  # wb
            nc.vector.tensor_tensor(wq(2), fy, wq(3), op=alu.subtract)  # wc
            nc.vector.tensor_scalar(out=uu, in0=fx, scalar1=-1.0, scalar2=1.0,
                                    op0=alu.mult, op1=alu.add)      # 1-fx
            nc.vector.tensor_tensor(wq(0), uu, wq(2), op=alu.subtract)  # wa

            dist = persist.tile([128, F], dt.float32)   # layout h*40 + k*4 + wc
            outx_sb = persist.tile([128, HW4], dt.float32)
            outy_sb = persist.tile([128, HW4], dt.float32)

            rightw_ap = rightw.ap()

            for h in range(HLOC):
                # gather: 5 calls x 1024 idxs -> G[p, j=(k*4+wc), 256]
                gidx_h = stream.tile([128, GH], dt.int16, tag="gidx")
                nc.sync.dma_start(out=gidx_h,
                                  in_=gidx.ap()[:, h * GH:(h + 1) * GH])
                G = gstream.tile([128, J, 256], dt.float16, tag="G")
                for c in range(NI // NIC):
                    nc.gpsimd.dma_gather(
                        out_ap=G[:, c * (NIC // 128):(c + 1) * (NIC // 128), :],
                        in_ap=rightw_ap,
                        idxs_ap=gidx_h[:, c * (NIC // 16):(c + 1) * (NIC // 16)],
                        num_idxs=NIC,
                        num_idxs_reg=NIC,
                        elem_size=256,
                    )
                # Act: broadcast weights to channel width, one op per y-pair
                # wTall layout: pair*2560 + j*64 + half*32 + c
                wTall = stream.tile([128, 2, J, 64], dt.float16, tag="wT")
                for pr in range(2):
                    nc.scalar.activation(
                        out=vw(wTall[:, pr, :, :], [[64, J], [C, 2], [1, C]]),
                        in_=vw(wquad[:, h * 4 * J + 2 * pr:(h + 1) * 4 * J],
                               [[4, J], [1, 2], [0, C]]),
                        func=mybir.ActivationFunctionType.Copy)
                # DVE: fused 4-corner multiply, pairwise adds, diff, reduce
                mAll = stream.tile([128, 2, J, 64], dt.float16, tag="mAll")
                nc.vector.tensor_tensor(
                    mAll[:, :, :, :],
                    vw(G[:, :, :], [[128, 2], [256, J], [1, 64]]),
                    wTall[:, :, :, :], op=alu.mult)
                sum12 = stream.tile([128, J, 64], dt.float16, tag="sum12")
                nc.vector.tensor_add(sum12, mAll[:, 0, :, :], mAll[:, 1, :, :])
                ss = stream.tile([128, J, C], dt.float16, tag="ss")
                nc.vector.tensor_add(ss, sum12[:, :, 0:C], sum12[:, :, C:2 * C])
                ee = stream.tile([128, J, C], dt.float16, tag="ee")
                nc.vector.tensor_tensor(
                    vw(ee[:, :, :], [[C * WC, K], [C, WC], [1, C]]),
                    vw(ss[:, :, :], [[C * WC, K], [C, WC], [1, C]]),
                    vw(left_sb[:, h * WC * C:(h + 1) * WC * C],
                       [[0, K], [C, WC], [1, C]]),
                    op=alu.subtract)
                nc.vector.tensor_reduce(
                    out=vw(dist[:, h * J:(h + 1) * J], [[1, J]]),
                    in_=ee, axis=mybir.AxisListType.X, op=alu.add,
                    apply_absolute_value=True)

                # ---- chunked softmax over K + weighted sums ----
                if h % CH == CH - 1:
                    c0 = (h // CH) * CH * J             # dist col offset
                    o0 = (h // CH) * CH * WC            # out col offset
                    dv = vw(dist[:, c0:c0 + CH * J],
                            [[J, CH], [1, WC], [WC, K]])
                    mt = stream.tile([128, CH * WC], dt.float32, tag="mt")
                    nc.vector.tensor_reduce(
                        out=vw(mt[:, :], [[WC, CH], [1, WC]]), in_=dv,
                        axis=mybir.AxisListType.X, op=alu.min)
                    q = stream.tile([128, CH * WC * K], dt.float32, tag="q")
                    qv = vw(q[:, :], [[WC * K, CH], [K, WC], [1, K]])
                    nc.vector.tensor_tensor(
                        qv, dv, vw(mt[:, :], [[WC, CH], [1, WC], [0, K]]),
                        op=alu.subtract)
                    pt = stream.tile([128, CH * WC * K], dt.float32, tag="pt")
                    nc.scalar.activation(out=pt, in_=q,
                                         func=mybir.ActivationFunctionType.Exp,
                                         scale=TEMP_SCALE)
                    ptv = vw(pt[:, :], [[WC * K, CH], [K, WC], [1, K]])
                    st = stream.tile([128, CH * WC], dt.float32, tag="st")
                    nc.vector.tensor_reduce(
                        out=vw(st[:, :], [[WC, CH], [1, WC]]), in_=ptv,
                        axis=mybir.AxisListType.X, op=alu.add)
                    rec = stream.tile([128, CH * WC], dt.float32, tag="rec")
                    nc.vector.reciprocal(rec, st)
                    for off_sb, osb, tg in ((offx_sb, outx_sb, "x"),
                                            (offy_sb, outy_sb, "y")):
                        ov = vw(off_sb[:, c0:c0 + CH * J],
                                [[J, CH], [1, WC], [WC, K]])
                        tx = stream.tile([128, CH * WC * K], dt.float32,
                                         tag=f"tx{tg}")
                        nc.vector.tensor_tensor(
                            vw(tx[:, :], [[WC * K, CH], [K, WC], [1, K]]),
                            ov, ptv, op=alu.mult)
                        nx = stream.tile([128, CH * WC], dt.float32,
                                         tag=f"nx{tg}")
                        nc.vector.tensor_reduce(
                            out=vw(nx[:, :], [[WC, CH], [1, WC]]),
                            in_=vw(tx[:, :], [[WC * K, CH], [K, WC], [1, K]]),
                            axis=mybir.AxisListType.X, op=alu.add)
                        nc.vector.tensor_mul(osb[:, o0:o0 + CH * WC], nx, rec)

            nc.sync.dma_start(out=outx.ap(), in_=outx_sb[:, :])
            nc.sync.dma_start(out=outy.ap(), in_=outy_sb[:, :])

    nc.compile()
    return nc


def _host_prep(left_features, right_features, offset_x, offset_y):
    """Per-core input dicts. All layout/addressing on host; lerp on device."""
    lf = np.asarray(left_features, np.float32)
    rf = np.asarray(right_features, np.float32)
    ox = np.asarray(offset_x, np.float32)
    oy = np.asarray(offset_y, np.float32)
    r_hwc = np.ascontiguousarray(rf[0].transpose(1, 2, 0))  # [H, W, C]
    l_hwc = lf[0].transpose(1, 2, 0)                        # [H, W, C]
    xs = np.arange(W, dtype=np.float32)

    in_maps = []
    metas = []
    for ci in range(NCORES):
        h0 = ci * HLOC
        ws = min(max(h0 - MARGIN, 0), H - WIN)
        rows = slice(h0, h0 + HLOC)

        # 64 window rows [ws-1, ws+63); row ws-1 is zeros at the global top
        win64 = np.zeros((WIN + 1, W, C), np.float32)
        lo = max(ws - 1, 0)
        win64[lo - (ws - 1):] = r_hwc[lo:ws + WIN]
        # fp16 padded image, cols -4..518; 4-parity patch buffers of
        # [2 rows x 4 px x 32 ch] elements, col_start = pi + 4e - 4
        pad = np.zeros((WIN + 1, 523, C), np.float16)
        pad[:, 4:4 + W] = win64.astype(np.float16)
        rightw = np.empty((4, PROWS, NE, 256), np.float16)
        for pi in range(4):
            Vp = pad[:, pi:pi + 4 * NE].reshape(WIN + 1, NE, 4, C)
            rightw[pi] = np.concatenate([Vp[:-1], Vp[1:]], axis=2).reshape(
                PROWS, NE, 256)
        rightw = rightw.reshape(-1, 256)

        # leftt [128, h*128 + wc*32 + c] fp16
        leftt = np.ascontiguousarray(
            l_hwc[rows].astype(np.float16).reshape(HLOC, WC, 128, C)
            .transpose(2, 0, 1, 3)).reshape(128, -1)

        # coords (f32 math identical to reference)
        oxs = ox[0, :, rows, :]
        oys = oy[0, :, rows, :]
        rx = np.clip(xs[None, None, :] - oxs, 0.0, np.float32(W - 1))
        hg = np.arange(h0, h0 + HLOC, dtype=np.float32)
        ry_loc = np.clip((hg[None, :, None] - ws) - oys,
                         np.float32(-ws), np.float32(H - 1 - ws))
        ixf = rx - np.float32(0.5)
        x0 = np.floor(ixf).astype(np.int32)                  # [-1, 510]
        fxh = (ixf - np.floor(ixf)).astype(np.float32)
        iyf = ry_loc - np.float32(0.5)
        y0 = np.floor(iyf).astype(np.int32)                  # window-local
        fyh = (iyf - np.floor(iyf)).astype(np.float32)
        r = np.clip(y0, -1, PROWS - 2) + 1                   # patch row [0, 62]
        pi = x0 & 3
        e = (x0 >> 2) + 1
        idx0 = ((pi * PROWS + r) * NE + e).astype(np.int16)  # [K, HLOC, W]

        def fold(a, dtp):
            return np.ascontiguousarray(
                a.reshape(K, HLOC, WC, 128).transpose(3, 1, 0, 2)
            ).reshape(128, -1).astype(dtp)

        # wrapped gidx layout [16, h, k, wc, g] replicated to 128 partitions
        gi = idx0.reshape(K, HLOC, WC, 8, 16).transpose(4, 1, 0, 2, 3)
        gi = np.ascontiguousarray(gi).reshape(16, -1)
        gidx_h = np.tile(gi, (8, 1))

        in_maps.append({
            "rightw": rightw, "leftt": leftt,
            "offx": fold(oxs, np.float16), "offy": fold(oys, np.float16),
            "fxw": fold(fxh, np.float16), "fyw": fold(fyh, np.float16),
            "gidx": gidx_h,
        })
        metas.append((h0, ws))
    return in_maps, metas


def _host_post(results, metas):
    ox = np.empty((1, 1, H, W), np.float32)
    oy = np.empty((1, 1, H, W), np.float32)
    for res, (h0, ws) in zip(results, metas):
        # outx free layout: chunk*32 + hh*4 + wc, partition = w % 128
        dx = res["outx"].reshape(128, NCH, CH, WC).transpose(1, 2, 3, 0)
        dy = res["outy"].reshape(128, NCH, CH, WC).transpose(1, 2, 3, 0)
        ox[0, 0, h0:h0 + HLOC] = dx.reshape(HLOC, W)
        oy[0, 0, h0:h0 + HLOC] = dy.reshape(HLOC, W)
    return ox, oy


def kernel(left_features, right_features, offset_x, offset_y):
    from concourse.bass_utils import run_bass_kernel_spmd

    assert left_features.shape == (B, C, H, W)
    in_maps, metas = _host_prep(left_features, right_features,
                                offset_x, offset_y)
    if "nc" not in _cache:
        _cache["nc"] = _build_bass()
    res = run_bass_kernel_spmd(_cache["nc"], in_maps, core_ids=list(range(NCORES)))
    return _host_post(res.results, metas)


# revision 14
# speedup vs baseline: 1.7248x; 1.0018x over previous
"""Trainium2 Bass kernel for nn_Evaluate_66735201845638.

Stereo-matching op: bilinear-sample right_features at K=10 per-pixel
(offset_x, offset_y) candidates, L1-compare against left_features over C=32
channels, sharp softmax (T=10000) over K, output expectation of the offsets.

Strategy (8 cores, rows sharded, 32 rows each), v3 (fp16 compute):
  - Host: packs right_features (fp16) into 4 x-parity patch buffers of 512B
    elements [2 rows x 4 px x 32 ch] over a 63-row halo window, computes
    int16 gather indices, fp16 lerp fractions, fp16 left features and
    offsets.
  - Device per output row h: five 1024-idx dma_gathers (Q7 scratch limit)
    fetch the per-sample corner patches (fp16, no cast needed); DVE builds
    interleaved corner weights once at startup; Act broadcasts them to
    channel width; DVE does the 4-corner weighted sum in fp16 2x mode with
    a single fused multiply over all corners, then abs-reduces over
    channels into dist. Every 8 rows a chunked softmax over K produces the
    weighted offset sums.
  - Host: stitches per-core [32, 512] outputs.

Self-contained: hardcodes B=1, C=32, H=256, W=512, K=10, 8 cores.
"""

import numpy as np

B, C, H, W, K = 1, 32, 256, 512, 10
NCORES = 8
HLOC = H // NCORES            # 32 output rows per core
MARGIN = 15                   # halo rows above/below (|offset_y| <= 14.5 safe)
WIN = HLOC + 2 * MARGIN + 1   # 63-row gather window
PROWS = WIN                   # 63 patch rows (r = y0_loc + 1 in [0, 62])
NE = 130                      # elements per (parity, patch row)
WC = W // 128                 # 4 column chunks of 128
NI = K * W                    # 5120 gather indices per row
NIC = 1024                    # indices per dma_gather call (Q7 scratch limit)
F = HLOC * K * WC             # 1280
J = K * WC                    # 40 sample groups per row
HW4 = HLOC * WC               # 128
CH = 8                        # rows per softmax chunk
NCH = HLOC // CH              # 4 chunks
TEMP_SCALE = -10000.0 / C

_cache = {}


def _build_bass():
    import concourse.bass as bass
    import concourse.bacc as bacc
    import concourse.tile as tile
    import concourse.mybir as mybir
    from concourse.mybir import AluOpType as alu

    dt = mybir.dt
    nc = bacc.Bacc("TRN2", target_bir_lowering=False, num_devices=NCORES)

    rightw = nc.dram_tensor("rightw", [4 * PROWS * NE, 256], dt.float16,
                            kind="ExternalInput")
    leftt = nc.dram_tensor("leftt", [128, HLOC * WC * C], dt.float16,
                           kind="ExternalInput")
    offx = nc.dram_tensor("offx", [128, F], dt.float16, kind="ExternalInput")
    offy = nc.dram_tensor("offy", [128, F], dt.float16, kind="ExternalInput")
    fxw = nc.dram_tensor("fxw", [128, F], dt.float16, kind="ExternalInput")
    fyw = nc.dram_tensor("fyw", [128, F], dt.float16, kind="ExternalInput")
    gidx = nc.dram_tensor("gidx", [128, HLOC * (NI // 16)], dt.int16,
                          kind="ExternalInput")
    outx = nc.dram_tensor("outx", [128, HW4], dt.float32, kind="ExternalOutput")
    outy = nc.dram_tensor("outy", [128, HW4], dt.float32, kind="ExternalOutput")

    def vw(sl, dims):
        """AP view: keep slice's partition dim + offset, replace free dims."""
        return bass.AP(tensor=sl.tensor, offset=sl.offset,
                       ap=[list(sl.ap[0])] + [list(d) for d in dims])

    GH = NI // 16   # 320 gidx columns per row

    with tile.TileContext(nc) as tc:
        with (
            tc.tile_pool(name="persist", bufs=1) as persist,
            tc.tile_pool(name="stream", bufs=2) as stream,
            tc.tile_pool(name="gstream", bufs=3) as gstream,
        ):
            fx = persist.tile([128, F], dt.float16)
            fy = persist.tile([128, F], dt.float16)
            left_sb = persist.tile([128, HLOC * WC * C], dt.float16)
            offx_sb = persist.tile([128, F], dt.float16)
            offy_sb = persist.tile([128, F], dt.float16)
            nc.sync.dma_start(out=fx, in_=fxw.ap())
            nc.sync.dma_start(out=fy, in_=fyw.ap())

            # chunk boundaries for softmax tails / chunked left loads
            CHS = [(0, 8), (8, 8), (16, 8), (24, 4), (28, 4)]
            tail_rows = {hs + n - 1: (hs, n) for hs, n in CHS}
            chunk_start = {hs: i for i, (hs, n) in enumerate(CHS)}

            def load_left_chunk(ci_):
                hs, n = CHS[ci_]
                lo, hi = hs * WC * C, (hs + n) * WC * C
                nc.sync.dma_start(out=left_sb[:, lo:hi],
                                  in_=leftt.ap()[:, lo:hi])

            load_left_chunk(0)

            # ---- interleaved corner weights wquad[j*4 + pair*2 + half] ----
            # order per sample j: [wa, wb, wc, wd] (y0x0, y0x1, y1x0, y1x1)
            wquad = persist.tile([128, 4 * F], dt.float16)
            uu = persist.tile([128, F], dt.float16)

            def wq(pos):
                return vw(wquad[:, pos:pos + 4 * F - 3], [[4, F]])

            nc.vector.tensor_tensor(wq(3), fx, fy, op=alu.mult)     # wd = fx*fy
            nc.vector.tensor_tensor(wq(1), fx, wq(3), op=alu.subtract)  # wb
            nc.vector.tensor_tensor(wq(2), fy, wq(3), op=alu.subtract)  # wc
            nc.vector.tensor_scalar(out=uu, in0=fx, scalar1=-1.0, scalar2=1.0,
                                    op0=alu.mult, op1=alu.add)      # 1-fx
            nc.vector.tensor_tensor(wq(0), uu, wq(2), op=alu.subtract)  # wa

            dist = persist.tile([128, F], dt.float32)   # layout h*40 + k*4 + wc
            outx_sb = persist.tile([128, HW4], dt.float32)
            outy_sb = persist.tile([128, HW4], dt.float32)

            rightw_ap = rightw.ap()

            for h in range(HLOC):
                # gather: 5 calls x 1024 idxs -> G[p, j=(k*4+wc), 256]
                gidx_h = stream.tile([128, GH], dt.int16, tag="gidx")
                nc.sync.dma_start(out=gidx_h,
                                  in_=gidx.ap()[:, h * GH:(h + 1) * GH])
                if h == 1:
                    nc.sync.dma_start(out=offx_sb, in_=offx.ap())
                    nc.sync.dma_start(out=offy_sb, in_=offy.ap())
                if h in chunk_start and chunk_start[h] + 1 < len(CHS):
                    load_left_chunk(chunk_start[h] + 1)
                G = gstream.tile([128, J, 256], dt.float16, tag="G")
                for c in range(NI // NIC):
                    nc.gpsimd.dma_gather(
                        out_ap=G[:, c * (NIC // 128):(c + 1) * (NIC // 128), :],
                        in_ap=rightw_ap,
                        idxs_ap=gidx_h[:, c * (NIC // 16):(c + 1) * (NIC // 16)],
                        num_idxs=NIC,
                        num_idxs_reg=NIC,
                        elem_size=256,
                    )
                # Act: broadcast weights to channel width, one op per y-pair
                # wTall layout: pair*2560 + j*64 + half*32 + c
                wTall = stream.tile([128, 2, J, 64], dt.float16, tag="wT")
                for pr in range(2):
                    nc.scalar.activation(
                        out=vw(wTall[:, pr, :, :], [[64, J], [C, 2], [1, C]]),
                        in_=vw(wquad[:, h * 4 * J + 2 * pr:(h + 1) * 4 * J],
                               [[4, J], [1, 2], [0, C]]),
                        func=mybir.ActivationFunctionType.Copy)
                # DVE: fused 4-corner multiply, pairwise adds, diff, reduce
                mAll = stream.tile([128, 2, J, 64], dt.float16, tag="mAll")
                nc.vector.tensor_tensor(
                    mAll[:, :, :, :],
                    vw(G[:, :, :], [[128, 2], [256, J], [1, 64]]),
                    wTall[:, :, :, :], op=alu.mult)
                sum12 = stream.tile([128, J, 64], dt.float16, tag="sum12")
                nc.vector.tensor_add(sum12, mAll[:, 0, :, :], mAll[:, 1, :, :])
                ss = stream.tile([128, J, C], dt.float16, tag="ss")
                nc.vector.tensor_add(ss, sum12[:, :, 0:C], sum12[:, :, C:2 * C])
                ee = stream.tile([128, J, C], dt.float16, tag="ee")
                nc.vector.tensor_tensor(
                    vw(ee[:, :, :], [[C * WC, K], [C, WC], [1, C]]),
                    vw(ss[:, :, :], [[C * WC, K], [C, WC], [1, C]]),
                    vw(left_sb[:, h * WC * C:(h + 1) * WC * C],
                       [[0, K], [C, WC], [1, C]]),
                    op=alu.subtract)
                nc.vector.tensor_reduce(
                    out=vw(dist[:, h * J:(h + 1) * J], [[1, J]]),
                    in_=ee, axis=mybir.AxisListType.X, op=alu.add,
                    apply_absolute_value=True)

                # ---- chunked softmax over K + weighted sums ----
                if h in tail_rows:
                    hs, n = tail_rows[h]
                    c0 = hs * J                          # dist col offset
                    o0 = hs * WC                         # out col offset
                    dv = vw(dist[:, c0:c0 + n * J],
                            [[J, n], [1, WC], [WC, K]])
                    mt = stream.tile([128, CH * WC], dt.float32, tag="mt")
                    nc.vector.tensor_reduce(
                        out=vw(mt[:, :], [[WC, n], [1, WC]]), in_=dv,
                        axis=mybir.AxisListType.X, op=alu.min)
                    q = stream.tile([128, CH * WC * K], dt.float32, tag="q")
                    qv = vw(q[:, :], [[WC * K, n], [K, WC], [1, K]])
                    nc.vector.tensor_tensor(
                        qv, dv, vw(mt[:, :], [[WC, n], [1, WC], [0, K]]),
                        op=alu.subtract)
                    pt = stream.tile([128, CH * WC * K], dt.float32, tag="pt")
                    nc.scalar.activation(out=pt[:, 0:n * WC * K],
                                         in_=q[:, 0:n * WC * K],
                                         func=mybir.ActivationFunctionType.Exp,
                                         scale=TEMP_SCALE)
                    ptv = vw(pt[:, :], [[WC * K, n], [K, WC], [1, K]])
                    st = stream.tile([128, CH * WC], dt.float32, tag="st")
                    nc.vector.tensor_reduce(
                        out=vw(st[:, :], [[WC, n], [1, WC]]), in_=ptv,
                        axis=mybir.AxisListType.X, op=alu.add)
                    rec = stream.tile([128, CH * WC], dt.float32, tag="rec")
                    nc.vector.reciprocal(rec[:, 0:n * WC], st[:, 0:n * WC])
                    for off_sb, osb, odr, tg in (
                            (offx_sb, outx_sb, outx, "x"),
                            (offy_sb, outy_sb, outy, "y")):
                        ov = vw(off_sb[:, c0:c0 + n * J],
                                [[J, n], [1, WC], [WC, K]])
                        tx = stream.tile([128, CH * WC * K], dt.float32,
                                         tag=f"tx{tg}")
                        nc.vector.tensor_tensor(
                            vw(tx[:, :], [[WC * K, n], [K, WC], [1, K]]),
                            ov, ptv, op=alu.mult)
                        nx = stream.tile([128, CH * WC], dt.float32,
                                         tag=f"nx{tg}")
                        nc.vector.tensor_reduce(
                            out=vw(nx[:, :], [[WC, n], [1, WC]]),
                            in_=vw(tx[:, :], [[WC * K, n], [K, WC], [1, K]]),
                            axis=mybir.AxisListType.X, op=alu.add)
                        nc.vector.tensor_mul(osb[:, o0:o0 + n * WC],
                                             nx[:, 0:n * WC],
                                             rec[:, 0:n * WC])
                        nc.sync.dma_start(
                            out=odr.ap()[:, o0:o0 + n * WC],
                            in_=osb[:, o0:o0 + n * WC])

    nc.compile()
    return nc


def _host_prep(left_features, right_features, offset_x, offset_y):
    """Per-core input dicts. All layout/addressing on host; lerp on device."""
    lf = np.asarray(left_features, np.float32)
    rf = np.asarray(right_features, np.float32)
    ox = np.asarray(offset_x, np.float32)
    oy = np.asarray(offset_y, np.float32)
    r_hwc = np.ascontiguousarray(rf[0].transpose(1, 2, 0))  # [H, W, C]
    l_hwc = lf[0].transpose(1, 2, 0)                        # [H, W, C]
    xs = np.arange(W, dtype=np.float32)

    in_maps = []
    metas = []
    for ci in range(NCORES):
        h0 = ci * HLOC
        ws = min(max(h0 - MARGIN, 0), H - WIN)
        rows = slice(h0, h0 + HLOC)

        # 64 window rows [ws-1, ws+63); row ws-1 is zeros at the global top
        win64 = np.zeros((WIN + 1, W, C), np.float32)
        lo = max(ws - 1, 0)
        win64[lo - (ws - 1):] = r_hwc[lo:ws + WIN]
        # fp16 padded image, cols -4..518; 4-parity patch buffers of
        # [2 rows x 4 px x 32 ch] elements, col_start = pi + 4e - 4
        pad = np.zeros((WIN + 1, 523, C), np.float16)
        pad[:, 4:4 + W] = win64.astype(np.float16)
        rightw = np.empty((4, PROWS, NE, 256), np.float16)
        for pi in range(4):
            Vp = pad[:, pi:pi + 4 * NE].reshape(WIN + 1, NE, 4, C)
            rightw[pi] = np.concatenate([Vp[:-1], Vp[1:]], axis=2).reshape(
                PROWS, NE, 256)
        rightw = rightw.reshape(-1, 256)

        # leftt [128, h*128 + wc*32 + c] fp16
        leftt = np.ascontiguousarray(
            l_hwc[rows].astype(np.float16).reshape(HLOC, WC, 128, C)
            .transpose(2, 0, 1, 3)).reshape(128, -1)

        # coords (f32 math identical to reference)
        oxs = ox[0, :, rows, :]
        oys = oy[0, :, rows, :]
        rx = np.clip(xs[None, None, :] - oxs, 0.0, np.float32(W - 1))
        hg = np.arange(h0, h0 + HLOC, dtype=np.float32)
        ry_loc = np.clip((hg[None, :, None] - ws) - oys,
                         np.float32(-ws), np.float32(H - 1 - ws))
        ixf = rx - np.float32(0.5)
        x0 = np.floor(ixf).astype(np.int32)                  # [-1, 510]
        fxh = (ixf - np.floor(ixf)).astype(np.float32)
        iyf = ry_loc - np.float32(0.5)
        y0 = np.floor(iyf).astype(np.int32)                  # window-local
        fyh = (iyf - np.floor(iyf)).astype(np.float32)
        r = np.clip(y0, -1, PROWS - 2) + 1                   # patch row [0, 62]
        pi = x0 & 3
        e = (x0 >> 2) + 1
        idx0 = ((pi * PROWS + r) * NE + e).astype(np.int16)  # [K, HLOC, W]

        def fold(a, dtp):
            return np.ascontiguousarray(
                a.reshape(K, HLOC, WC, 128).transpose(3, 1, 0, 2)
            ).reshape(128, -1).astype(dtp)

        # wrapped gidx layout [16, h, k, wc, g] replicated to 128 partitions
        gi = idx0.reshape(K, HLOC, WC, 8, 16).transpose(4, 1, 0, 2, 3)
        gi = np.ascontiguousarray(gi).reshape(16, -1)
        gidx_h = np.tile(gi, (8, 1))

        in_maps.append({
            "rightw": rightw, "leftt": leftt,
            "offx": fold(oxs, np.float16), "offy": fold(oys, np.float16),
            "fxw": fold(fxh, np.float16), "fyw": fold(fyh, np.float16),
            "gidx": gidx_h,
        })
        metas.append((h0, ws))
    return in_maps, metas


def _host_post(results, metas):
    ox = np.empty((1, 1, H, W), np.float32)
    oy = np.empty((1, 1, H, W), np.float32)
    for res, (h0, ws) in zip(results, metas):
        # outx free layout: chunk*32 + hh*4 + wc, partition = w % 128
        dx = res["outx"].reshape(128, NCH, CH, WC).transpose(1, 2, 3, 0)
        dy = res["outy"].reshape(128, NCH, CH, WC).transpose(1, 2, 3, 0)
        ox[0, 0, h0:h0 + HLOC] = dx.reshape(HLOC, W)
        oy[0, 0, h0:h0 + HLOC] = dy.reshape(HLOC, W)
    return ox, oy


def kernel(left_features, right_features, offset_x, offset_y):
    from concourse.bass_utils import run_bass_kernel_spmd

    assert left_features.shape == (B, C, H, W)
    in_maps, metas = _host_prep(left_features, right_features,
                                offset_x, offset_y)
    if "nc" not in _cache:
        _cache["nc"] = _build_bass()
    res = run_bass_kernel_spmd(_cache["nc"], in_maps, core_ids=list(range(NCORES)))
    return _host_post(res.results, metas)


# revision 35
# speedup vs baseline: 1.7303x; 1.0032x over previous
"""Trainium2 Bass kernel for nn_Evaluate_66735201845638.

Stereo-matching op: bilinear-sample right_features at K=10 per-pixel
(offset_x, offset_y) candidates, L1-compare against left_features over C=32
channels, sharp softmax (T=10000) over K, output expectation of the offsets.

Strategy (8 cores, rows sharded, 32 rows each), v3 (fp16 compute):
  - Host: packs right_features (fp16) into 4 x-parity patch buffers of 512B
    elements [2 rows x 4 px x 32 ch] over a 63-row halo window, computes
    int16 gather indices, fp16 lerp fractions, fp16 left features and
    offsets.
  - Device per output row h: five 1024-idx dma_gathers (Q7 scratch limit)
    fetch the per-sample corner patches (fp16, no cast needed); DVE builds
    interleaved corner weights once at startup; Act broadcasts them to
    channel width; DVE does the 4-corner weighted sum in fp16 2x mode with
    a single fused multiply over all corners, then abs-reduces over
    channels into dist. Every 8 rows a chunked softmax over K produces the
    weighted offset sums.
  - Host: stitches per-core [32, 512] outputs.

Self-contained: hardcodes B=1, C=32, H=256, W=512, K=10, 8 cores.
"""

import numpy as np

B, C, H, W, K = 1, 32, 256, 512, 10
NCORES = 8
HLOC = H // NCORES            # 32 output rows per core
MARGIN = 15                   # halo rows above/below (|offset_y| <= 14.5 safe)
WIN = HLOC + 2 * MARGIN + 1   # 63-row gather window
PROWS = WIN                   # 63 patch rows (r = y0_loc + 1 in [0, 62])
NE = 130                      # elements per (parity, patch row)
WC = W // 128                 # 4 column chunks of 128
NI = K * W                    # 5120 gather indices per row
NIC = 1024                    # indices per dma_gather call (Q7 scratch limit)
F = HLOC * K * WC             # 1280
J = K * WC                    # 40 sample groups per row
HW4 = HLOC * WC               # 128
CH = 8                        # rows per softmax chunk
NCH = HLOC // CH              # 4 chunks
TEMP_SCALE = -10000.0 / C

_cache = {}


def _build_bass():
    import concourse.bass as bass
    import concourse.bacc as bacc
    import concourse.tile as tile
    import concourse.mybir as mybir
    from concourse.mybir import AluOpType as alu

    dt = mybir.dt
    nc = bacc.Bacc("TRN2", target_bir_lowering=False, num_devices=NCORES)

    rightw = nc.dram_tensor("rightw", [4 * PROWS * NE, 256], dt.float16,
                            kind="ExternalInput")
    leftt = nc.dram_tensor("leftt", [128, HLOC * WC * C], dt.float16,
                           kind="ExternalInput")
    offx = nc.dram_tensor("offx", [128, F], dt.float16, kind="ExternalInput")
    offy = nc.dram_tensor("offy", [128, F], dt.float16, kind="ExternalInput")
    fxw = nc.dram_tensor("fxw", [128, F], dt.float16, kind="ExternalInput")
    fyw = nc.dram_tensor("fyw", [128, F], dt.float16, kind="ExternalInput")
    gidx = nc.dram_tensor("gidx", [128, HLOC * (NI // 16)], dt.int16,
                          kind="ExternalInput")
    outx = nc.dram_tensor("outx", [128, HW4], dt.float32, kind="ExternalOutput")
    outy = nc.dram_tensor("outy", [128, HW4], dt.float32, kind="ExternalOutput")

    def vw(sl, dims):
        """AP view: keep slice's partition dim + offset, replace free dims."""
        return bass.AP(tensor=sl.tensor, offset=sl.offset,
                       ap=[list(sl.ap[0])] + [list(d) for d in dims])

    GH = NI // 16   # 320 gidx columns per row

    with tile.TileContext(nc) as tc:
        with (
            tc.tile_pool(name="persist", bufs=1) as persist,
            tc.tile_pool(name="stream", bufs=2) as stream,
            tc.tile_pool(name="gstream", bufs=3) as gstream,
            tc.tile_pool(name="gxpool", bufs=4) as gxpool,
        ):
            fx = persist.tile([128, F], dt.float16)
            fy = persist.tile([128, F], dt.float16)
            left_sb = persist.tile([128, HLOC * WC * C], dt.float16)
            offx_sb = persist.tile([128, F], dt.float16)
            offy_sb = persist.tile([128, F], dt.float16)

            # chunk boundaries for softmax tails / chunked left loads
            CHS = [(0, 8), (8, 8), (16, 8), (24, 8)]
            tail_rows = {hs + n - 1: (hs, n) for hs, n in CHS}
            chunk_start = {hs: i for i, (hs, n) in enumerate(CHS)}

            def load_left_chunk(ci_):
                hs, n = CHS[ci_]
                lo, hi = hs * WC * C, (hs + n) * WC * C
                nc.sync.dma_start(out=left_sb[:, lo:hi],
                                  in_=leftt.ap()[:, lo:hi])



            # ---- interleaved corner weights wquad[j*4 + pair*2 + half] ----
            # order per sample j: [wa, wb, wc, wd] (y0x0, y0x1, y1x0, y1x1)
            wquad = persist.tile([128, 4 * F], dt.float16)
            uu = persist.tile([128, F], dt.float16)

            def wq(pos):
                return vw(wquad[:, pos:pos + 4 * F - 3], [[4, F]])

            def weight_prep():
                nc.vector.tensor_tensor(wq(3), fx, fy, op=alu.mult)  # wd
                nc.vector.tensor_tensor(wq(1), fx, wq(3), op=alu.subtract)
                nc.vector.tensor_tensor(wq(2), fy, wq(3), op=alu.subtract)
                nc.vector.tensor_scalar(out=uu, in0=fx, scalar1=-1.0,
                                        scalar2=1.0, op0=alu.mult,
                                        op1=alu.add)                 # 1-fx
                nc.vector.tensor_tensor(wq(0), uu, wq(2), op=alu.subtract)

            dist = persist.tile([128, F], dt.float32)   # layout h*40 + k*4 + wc
            outx_sb = persist.tile([128, HW4], dt.float32)
            outy_sb = persist.tile([128, HW4], dt.float32)

            rightw_ap = rightw.ap()

            def emit_compute(h, G, wTall, mAll, j0, j1):
                """Weighted 4-corner sum + diff + abs-reduce for j in [j0, j1).
                G's first free index is j-local (G row 0 == sample j0)."""
                nj = j1 - j0
                nc.vector.tensor_tensor(
                    vw(mAll[:, :, j0:j1, :], [[J * 64, 2], [64, nj], [1, 64]]),
                    vw(G[:, 0:nj, :], [[128, 2], [256, nj], [1, 64]]),
                    vw(wTall[:, :, j0:j1, :], [[J * 64, 2], [64, nj], [1, 64]]),
                    op=alu.mult)
                sum12 = stream.tile([128, J, 64], dt.float16, tag="sum12")
                nc.vector.tensor_add(sum12[:, j0:j1, :], mAll[:, 0, j0:j1, :],
                                     mAll[:, 1, j0:j1, :])
                ss = stream.tile([128, J, C], dt.float16, tag="ss")
                nc.vector.tensor_add(ss[:, j0:j1, :], sum12[:, j0:j1, 0:C],
                                     sum12[:, j0:j1, C:2 * C])
                ee = stream.tile([128, J, C], dt.float16, tag="ee")
                nc.vector.tensor_tensor(
                    vw(ee[:, j0:j1, :], [[C, nj], [1, C]]),
                    vw(ss[:, j0:j1, :], [[C, nj], [1, C]]),
                    vw(left_sb[:, h * WC * C:(h + 1) * WC * C],
                       [[0, (j1 - j0) // WC], [C, WC], [1, C]]),
                    op=alu.subtract)
                nc.vector.tensor_reduce(
                    out=vw(dist[:, h * J + j0:h * J + j1], [[1, nj]]),
                    in_=ee[:, j0:j1, :], axis=mybir.AxisListType.X, op=alu.add,
                    apply_absolute_value=True)

            for h in range(HLOC):
                # gather: 5 calls x 1024 idxs -> G[p, j=(k*4+wc), 256]
                gidx_h = gxpool.tile([128, GH], dt.int16, tag="gidx")
                nc.sync.dma_start(out=gidx_h,
                                  in_=gidx.ap()[:, h * GH:(h + 1) * GH])
                if h == 0:
                    nc.sync.dma_start(out=fx, in_=fxw.ap())
                    nc.sync.dma_start(out=fy, in_=fyw.ap())
                    weight_prep()
                    load_left_chunk(0)
                G = gstream.tile([128, J, 256], dt.float16, tag="G")
                parts = [(G, 0, J, range(0, 5))]
                # Act: broadcast weights to channel width, one op per y-pair
                # wTall layout: pair*2560 + j*64 + half*32 + c
                wTall = stream.tile([128, 2, J, 64], dt.float16, tag="wT")
                for pr in range(2):
                    nc.scalar.activation(
                        out=vw(wTall[:, pr, :, :], [[64, J], [C, 2], [1, C]]),
                        in_=vw(wquad[:, h * 4 * J + 2 * pr:(h + 1) * 4 * J],
                               [[4, J], [1, 2], [0, C]]),
                        func=mybir.ActivationFunctionType.Copy)
                mAll = stream.tile([128, 2, J, 64], dt.float16, tag="mAll")
                for Gt, j0, j1, crange in parts:
                    for c in crange:
                        nc.gpsimd.dma_gather(
                            out_ap=Gt[:, c * (NIC // 128) - j0:
                                      (c + 1) * (NIC // 128) - j0, :],
                            in_ap=rightw_ap,
                            idxs_ap=gidx_h[:, c * (NIC // 16):
                                           (c + 1) * (NIC // 16)],
                            num_idxs=NIC,
                            num_idxs_reg=NIC,
                            elem_size=256,
                        )
                if h == 3:
                    nc.sync.dma_start(out=offx_sb, in_=offx.ap())
                    nc.sync.dma_start(out=offy_sb, in_=offy.ap())
                if h - 2 in chunk_start and chunk_start[h - 2] + 1 < len(CHS):
                    load_left_chunk(chunk_start[h - 2] + 1)
                for Gt, j0, j1, crange in parts:
                    emit_compute(h, Gt, wTall, mAll, j0, j1)

                # ---- chunked softmax over K + weighted sums ----
                if h in tail_rows:
                    hs, n = tail_rows[h]
                    c0 = hs * J                          # dist col offset
                    o0 = hs * WC                         # out col offset
                    dv = vw(dist[:, c0:c0 + n * J],
                            [[J, n], [1, WC], [WC, K]])
                    mt = stream.tile([128, CH * WC], dt.float32, tag="mt")
                    nc.vector.tensor_reduce(
                        out=vw(mt[:, :], [[WC, n], [1, WC]]), in_=dv,
                        axis=mybir.AxisListType.X, op=alu.min)
                    q = stream.tile([128, CH * WC * K], dt.float32, tag="q")
                    qv = vw(q[:, :], [[WC * K, n], [K, WC], [1, K]])
                    nc.vector.tensor_tensor(
                        qv, dv, vw(mt[:, :], [[WC, n], [1, WC], [0, K]]),
                        op=alu.subtract)
                    pt = stream.tile([128, CH * WC * K], dt.float32, tag="pt")
                    nc.scalar.activation(out=pt[:, 0:n * WC * K],
                                         in_=q[:, 0:n * WC * K],
                                         func=mybir.ActivationFunctionType.Exp,
                                         scale=TEMP_SCALE)
                    ptv = vw(pt[:, :], [[WC * K, n], [K, WC], [1, K]])
                    st = stream.tile([128, CH * WC], dt.float32, tag="st")
                    nc.vector.tensor_reduce(
                        out=vw(st[:, :], [[WC, n], [1, WC]]), in_=ptv,
                        axis=mybir.AxisListType.X, op=alu.add)
                    rec = stream.tile([128, CH * WC], dt.float32, tag="rec")
                    nc.vector.reciprocal(rec[:, 0:n * WC], st[:, 0:n * WC])
                    for off_sb, osb, odr, tg in (
                            (offx_sb, outx_sb, outx, "x"),
                            (offy_sb, outy_sb, outy, "y")):
                        ov = vw(off_sb[:, c0:c0 + n * J],
                                [[J, n], [1, WC], [WC, K]])
                        tx = stream.tile([128, CH * WC * K], dt.float32,
                                         tag=f"tx{tg}")
                        nc.vector.tensor_tensor(
                            vw(tx[:, :], [[WC * K, n], [K, WC], [1, K]]),
                            ov, ptv, op=alu.mult)
                        nx = stream.tile([128, CH * WC], dt.float32,
                                         tag=f"nx{tg}")
                        nc.vector.tensor_reduce(
                            out=vw(nx[:, :], [[WC, n], [1, WC]]),
                            in_=vw(tx[:, :], [[WC * K, n], [K, WC], [1, K]]),
                            axis=mybir.AxisListType.X, op=alu.add)
                        nc.vector.tensor_mul(osb[:, o0:o0 + n * WC],
                                             nx[:, 0:n * WC],
                                             rec[:, 0:n * WC])
                        nc.sync.dma_start(
                            out=odr.ap()[:, o0:o0 + n * WC],
                            in_=osb[:, o0:o0 + n * WC])

    nc.compile()
    return nc


def _host_prep(left_features, right_features, offset_x, offset_y):
    """Per-core input dicts. All layout/addressing on host; lerp on device."""
    lf = np.asarray(left_features, np.float32)
    rf = np.asarray(right_features, np.float32)
    ox = np.asarray(offset_x, np.float32)
    oy = np.asarray(offset_y, np.float32)
    r_hwc = np.ascontiguousarray(rf[0].transpose(1, 2, 0))  # [H, W, C]
    l_hwc = lf[0].transpose(1, 2, 0)                        # [H, W, C]
    xs = np.arange(W, dtype=np.float32)

    in_maps = []
    metas = []
    for ci in range(NCORES):
        h0 = ci * HLOC
        ws = min(max(h0 - MARGIN, 0), H - WIN)
        rows = slice(h0, h0 + HLOC)

        # 64 window rows [ws-1, ws+63); row ws-1 is zeros at the global top
        win64 = np.zeros((WIN + 1, W, C), np.float32)
        lo = max(ws - 1, 0)
        win64[lo - (ws - 1):] = r_hwc[lo:ws + WIN]
        # fp16 padded image, cols -4..518; 4-parity patch buffers of
        # [2 rows x 4 px x 32 ch] elements, col_start = pi + 4e - 4
        pad = np.zeros((WIN + 1, 523, C), np.float16)
        pad[:, 4:4 + W] = win64.astype(np.float16)
        rightw = np.empty((4, PROWS, NE, 256), np.float16)
        for pi in range(4):
            Vp = pad[:, pi:pi + 4 * NE].reshape(WIN + 1, NE, 4, C)
            rightw[pi] = np.concatenate([Vp[:-1], Vp[1:]], axis=2).reshape(
                PROWS, NE, 256)
        rightw = rightw.reshape(-1, 256)

        # leftt [128, h*128 + wc*32 + c] fp16
        leftt = np.ascontiguousarray(
            l_hwc[rows].astype(np.float16).reshape(HLOC, WC, 128, C)
            .transpose(2, 0, 1, 3)).reshape(128, -1)

        # coords (f32 math identical to reference)
        oxs = ox[0, :, rows, :]
        oys = oy[0, :, rows, :]
        rx = np.clip(xs[None, None, :] - oxs, 0.0, np.float32(W - 1))
        hg = np.arange(h0, h0 + HLOC, dtype=np.float32)
        ry_loc = np.clip((hg[None, :, None] - ws) - oys,
                         np.float32(-ws), np.float32(H - 1 - ws))
        ixf = rx - np.float32(0.5)
        x0 = np.floor(ixf).astype(np.int32)                  # [-1, 510]
        fxh = (ixf - np.floor(ixf)).astype(np.float32)
        iyf = ry_loc - np.float32(0.5)
        y0 = np.floor(iyf).astype(np.int32)                  # window-local
        fyh = (iyf - np.floor(iyf)).astype(np.float32)
        r = np.clip(y0, -1, PROWS - 2) + 1                   # patch row [0, 62]
        pi = x0 & 3
        e = (x0 >> 2) + 1
        idx0 = ((pi * PROWS + r) * NE + e).astype(np.int16)  # [K, HLOC, W]

        def fold(a, dtp):
            return np.ascontiguousarray(
                a.reshape(K, HLOC, WC, 128).transpose(3, 1, 0, 2)
            ).reshape(128, -1).astype(dtp)

        # wrapped gidx layout [16, h, k, wc, g] replicated to 128 partitions
        gi = idx0.reshape(K, HLOC, WC, 8, 16).transpose(4, 1, 0, 2, 3)
        gi = np.ascontiguousarray(gi).reshape(16, -1)
        gidx_h = np.tile(gi, (8, 1))

        in_maps.append({
            "rightw": rightw, "leftt": leftt,
            "offx": fold(oxs, np.float16), "offy": fold(oys, np.float16),
            "fxw": fold(fxh, np.float16), "fyw": fold(fyh, np.float16),
            "gidx": gidx_h,
        })
        metas.append((h0, ws))
    return in_maps, metas


def _host_post(results, metas):
    ox = np.empty((1, 1, H, W), np.float32)
    oy = np.empty((1, 1, H, W), np.float32)
    for res, (h0, ws) in zip(results, metas):
        # outx free layout: chunk*32 + hh*4 + wc, partition = w % 128
        dx = res["outx"].reshape(128, NCH, CH, WC).transpose(1, 2, 3, 0)
        dy = res["outy"].reshape(128, NCH, CH, WC).transpose(1, 2, 3, 0)
        ox[0, 0, h0:h0 + HLOC] = dx.reshape(HLOC, W)
        oy[0, 0, h0:h0 + HLOC] = dy.reshape(HLOC, W)
    return ox, oy


def kernel(left_features, right_features, offset_x, offset_y):
    from concourse.bass_utils import run_bass_kernel_spmd

    assert left_features.shape == (B, C, H, W)
    in_maps, metas = _host_prep(left_features, right_features,
                                offset_x, offset_y)
    if "nc" not in _cache:
        _cache["nc"] = _build_bass()
    res = run_bass_kernel_spmd(_cache["nc"], in_maps, core_ids=list(range(NCORES)))
    return _host_post(res.results, metas)
